# revision 5
# baseline (speedup 1.0000x reference)
"""DictionaryConv1D Trainium2 kernel.

reference:
  sparse = conv1d(x, dictionary, pad=4)        # [B, 64, L], 9-tap
  feat   = relu(w1 @ sparse + b1)              # [B, 256, L]
  out    = w2 @ feat + b2                      # [B, 64, L]

Strategy: data-parallel over batch (32 rows -> 4 per core on 8 cores).
Per core, the conv is 5 PSUM-accumulated matmuls per 512-column tile: the 9
taps are packed in pairs into a K=128 contraction (two 1-shifted copies of x
stacked on partitions 0-63 / 64-127), so each matmul costs the same N columns
as a K=64 one would. The 1x1 convs are plain matmuls over the tile. Matmuls
run in float32r; everything else fp32.
"""
import sys

sys.path.insert(0, "/opt/trn_rl_repo")

import numpy as np
from contextlib import ExitStack

import concourse.bass as bass
import concourse.mybir as mybir
import concourse.tile as tile
from concourse.vector_clock import ScopedClock
from concourse.bass_utils import run_bass_kernel_spmd

B, C_IN, L = 32, 64, 8192
A, C_OUT, KTAPS = 64, 256, 9
NCORES = 8
BPC = B // NCORES          # batch rows per core
LPAD = L + 10              # 4 left pad + 5 right pad + 1 shift spare
NT = 512                   # L-tile columns (one PSUM bank)
NTILES = L // NT
NPAIRS = 5                 # ceil(9/2) tap pairs
f32 = mybir.dt.float32
f32r = mybir.dt.float32r

USE_F32R = True


class _SplitDrainTileContext(tile.TileContext):
    """This walrus build rejects instructions carrying >1 sync wait. Split
    every multi-wait instruction's extra waits onto same-engine nops placed
    immediately before it (same-engine program order makes this equivalent),
    and do the same for the Tile epilogue drain."""

    def _split_multi_waits(self):
        nc = self.nc
        for fn in nc.m.functions:
            for bb in fn.blocks:
                insts = list(bb.instructions)
                out = []
                changed = False
                for inst in insts:
                    si = inst.sync_info
                    if si is not None and len(si.on_wait) > 1:
                        ow = list(si.on_wait)
                        for w in ow[:-1]:
                            nop = mybir.InstNoOp(
                                name=nc.get_next_instruction_name(),
                                engine=inst.engine,
                                sync_info=mybir.SyncInfo(
                                    on_wait=[w], on_update=[]
                                ),
                                bass_nofuse=True,
                            )
                            nc.register_instruction(nop)
                            out.append(nop)
                        si.on_wait = ow[-1:]
                        inst.sync_info = si
                        changed = True
                    out.append(inst)
                if changed:
                    bb.instructions = out

    def _drain_and_barrier(self, tick_clock, wait_clock):
        probe = self.nc.sync.nop(nofuse=True)
        wait_clock.add_sem_waits(
            probe.ins, ScopedClock({None: tick_clock.global_clock})
        )
        si = probe.ins.sync_info
        if si is not None:
            ow = list(si.on_wait)
            if len(ow) > 1:
                si.on_wait = ow[:1]
                probe.ins.sync_info = si
                for w in ow[1:]:
                    extra = self.nc.sync.nop(nofuse=True)
                    extra.ins.sync_info = mybir.SyncInfo(on_wait=[w], on_update=[])
        self.nc.sync.drain()
        self._split_multi_waits()

        self.nc.all_engine_barrier()
        assert self.sems is not None
        popped = self.nc._tile_sem_poison_stack.pop()
        assert popped is self._sem_poison
        self.nc.clear_and_free_semaphores(list(self.sems.allocated().values()))
        self.nc.all_engine_barrier()


def _mm(ap):
    return ap


def _build_nc():
    nc = bass.Bass("TRN2", target_bir_lowering=False, debug=False,
                   num_devices=NCORES)
    xin = nc.dram_tensor("x", [BPC, C_IN, LPAD], f32r, kind="ExternalInput").ap()
    dl = nc.dram_tensor("dl", [128, NPAIRS * A], f32r, kind="ExternalInput").ap()
    w1t = nc.dram_tensor("w1t", [A, C_OUT], f32r, kind="ExternalInput").ap()
    w2t = nc.dram_tensor("w2t", [C_OUT, C_IN], f32r, kind="ExternalInput").ap()
    b1d = nc.dram_tensor("b1", [128, 2], f32, kind="ExternalInput").ap()
    b2d = nc.dram_tensor("b2", [C_IN, 1], f32, kind="ExternalInput").ap()
    y = nc.dram_tensor("y", [BPC, C_IN, L], f32, kind="ExternalOutput").ap()

    Relu = mybir.ActivationFunctionType.Relu
    Ident = mybir.ActivationFunctionType.Identity
    add = mybir.AluOpType.add
    amax = mybir.AluOpType.max

    with _SplitDrainTileContext(nc) as tc, ExitStack() as ctx:
        consts = ctx.enter_context(tc.tile_pool(name="consts", bufs=1))
        xrow_p = ctx.enter_context(tc.tile_pool(name="xrow", bufs=2))
        orow_p = ctx.enter_context(tc.tile_pool(name="orow", bufs=2))
        sp_p = ctx.enter_context(tc.tile_pool(name="sp", bufs=3))
        ft_p = ctx.enter_context(tc.tile_pool(name="ft", bufs=3))
        psA = ctx.enter_context(tc.tile_pool(name="psA", bufs=2, space="PSUM"))
        psB = ctx.enter_context(tc.tile_pool(name="psB", bufs=2, space="PSUM"))
        psC = ctx.enter_context(tc.tile_pool(name="psC", bufs=2, space="PSUM"))

        dl_sb = consts.tile([128, NPAIRS * A], f32r, tag="dl")
        nc.sync.dma_start(dl_sb[:], dl[:])
        w1t_sb = consts.tile([A, C_OUT], f32r, tag="w1t")
        nc.sync.dma_start(w1t_sb[:], w1t[:])
        w2t_sb = consts.tile([128, 2 * C_IN], f32r, tag="w2t")
        nc.sync.dma_start(w2t_sb[:, 0:C_IN], w2t[0:128, :])
        nc.sync.dma_start(w2t_sb[:, C_IN:2 * C_IN], w2t[128:256, :])
        b1_sb = consts.tile([128, 2], f32, tag="b1")
        nc.sync.dma_start(b1_sb[:], b1d[:])
        b2_sb = consts.tile([C_IN, 1], f32, tag="b2")
        nc.sync.dma_start(b2_sb[:], b2d[:])

        # software pipeline with a 1-tile skew: stage1(i) = conv+evac,
        # stage2(i) = mm1+relu+mm2+evac, issued as s1(i+1); s2(i)
        work = [(b, t) for b in range(BPC) for t in range(NTILES)]
        xts = {}
        orows = {}
        stage1_out = {}

        def stage1(i):
            b, t = work[i]
            if t == 0:
                xt = xrow_p.tile([128, LPAD], f32r, tag="xt", name=f"xt{b}")
                # partitions 0-63: x_pad; 64-127: x_pad shifted left by 1
                nc.sync.dma_start(xt[0:64, 0:LPAD - 1], xin[b, :, 0:LPAD - 1])
                nc.sync.dma_start(xt[64:128, 0:LPAD - 1], xin[b, :, 1:LPAD])
                xts[b] = xt
                orows[b] = orow_p.tile([C_IN, L], f32, tag="orow", name=f"orow{b}")
            xt = xts[b]
            l0 = NT * t
            cps = psA.tile([A, NT], f32, tag="cps")
            for p in range(NPAIRS):
                nc.tensor.matmul(
                    cps[:],
                    _mm(dl_sb[:, A * p:A * (p + 1)]),
                    _mm(xt[:, l0 + 2 * p:l0 + 2 * p + NT]),
                    start=(p == 0),
                    stop=(p == NPAIRS - 1),
                )
            sp = sp_p.tile([A, NT], f32r, tag="sp")
            nc.vector.tensor_copy(sp[:], cps[:])
            stage1_out[i] = sp

        def stage2(i):
            b, t = work[i]
            sp = stage1_out.pop(i)
            l0 = NT * t
            f0 = psB.tile([128, NT], f32, tag="f0")
            nc.tensor.matmul(f0[:], _mm(w1t_sb[:, 0:128]), _mm(sp[:]),
                             start=True, stop=True)
            f1 = psB.tile([128, NT], f32, tag="f1")
            nc.tensor.matmul(f1[:], _mm(w1t_sb[:, 128:256]), _mm(sp[:]),
                             start=True, stop=True)
            ft0 = ft_p.tile([128, NT], f32r, tag="ft0")
            nc.scalar.activation(ft0[:], f0[:], Relu, bias=b1_sb[:, 0:1])
            ft1 = ft_p.tile([128, NT], f32r, tag="ft1")
            nc.vector.tensor_scalar(ft1[:], f1[:], b1_sb[:, 1:2], 0.0, add, amax)
            ops = psC.tile([C_IN, NT], f32, tag="ops")
            nc.tensor.matmul(ops[:], _mm(w2t_sb[:, 0:C_IN]), _mm(ft0[:]),
                             start=True, stop=False)
            nc.tensor.matmul(ops[:], _mm(w2t_sb[:, C_IN:2 * C_IN]), _mm(ft1[:]),
                             start=False, stop=True)
            nc.scalar.activation(orows[b][:, l0:l0 + NT], ops[:], Ident,
                                 bias=b2_sb[:, 0:1])
            if t == NTILES - 1:
                nc.sync.dma_start(y[b], orows[b][:])

        n = len(work)
        stage1(0)
        for i in range(n):
            if i + 1 < n:
                stage1(i + 1)
            stage2(i)

    return nc


_NC = None


def _get_nc():
    global _NC
    if _NC is None:
        _NC = _build_nc()
    return _NC


def _prep_inputs(x, dictionary, w1, b1, w2, b2):
    x_pad = np.zeros((B, C_IN, LPAD), dtype=np.float32)
    x_pad[:, :, 4:4 + L] = x

    dl = np.zeros((128, NPAIRS, A), dtype=np.float32)
    for p in range(NPAIRS):
        dl[0:64, p, :] = dictionary[:, :, 2 * p].T
        if 2 * p + 1 < KTAPS:
            dl[64:128, p, :] = dictionary[:, :, 2 * p + 1].T
    dl = np.ascontiguousarray(dl.reshape(128, NPAIRS * A))

    w1t = np.ascontiguousarray(w1.T)                      # [A, C_OUT]
    w2t = np.ascontiguousarray(w2.T)                      # [C_OUT, C_IN]
    b1m = np.ascontiguousarray(b1.reshape(2, 128).T)      # [128, 2]
    b2m = np.ascontiguousarray(b2.reshape(C_IN, 1))

    shared = {"dl": dl, "w1t": w1t, "w2t": w2t, "b1": b1m, "b2": b2m}
    in_maps = []
    for c in range(NCORES):
        m = dict(shared)
        m["x"] = np.ascontiguousarray(x_pad[c * BPC:(c + 1) * BPC])
        in_maps.append(m)
    return in_maps


def run(inputs, **kwargs):
    """Run on hardware; returns (out [B, C_IN, L], BassKernelResults)."""
    arrs = {k: np.asarray(v, dtype=np.float32) for k, v in inputs.items()}
    in_maps = _prep_inputs(arrs["x"], arrs["dictionary"], arrs["w1"],
                           arrs["b1"], arrs["w2"], arrs["b2"])
    res = run_bass_kernel_spmd(_get_nc(), in_maps,
                               core_ids=list(range(NCORES)), **kwargs)
    out = np.concatenate([res.results[c]["y"] for c in range(NCORES)], axis=0)
    return out, res


def kernel(**inputs):
    out, _ = run(inputs)
    return out


# revision 18
# speedup vs baseline: 14.1307x; 14.1307x over previous
"""DictionaryConv1D Trainium2 kernel.

reference:
  sparse = conv1d(x, dictionary, pad=4)        # [B, 64, L], 9-tap
  feat   = relu(w1 @ sparse + b1)              # [B, 256, L]
  out    = w2 @ feat + b2                      # [B, 64, L]

Strategy: data-parallel over batch (32 rows -> 4 per core on 8 cores).
Per core, the conv is 5 PSUM-accumulated matmuls per 512-column tile: the 9
taps are packed in pairs into a K=128 contraction (two 1-shifted copies of x
stacked on partitions 0-63 / 64-127), so each matmul costs the same N columns
as a K=64 one would. The 1x1 convs are plain matmuls over the tile. Matmuls
run in float32r; everything else fp32.
"""
import sys

sys.path.insert(0, "/opt/trn_rl_repo")

import numpy as np
from contextlib import ExitStack

import concourse.bass as bass
import concourse.mybir as mybir
import concourse.tile as tile
from concourse.vector_clock import ScopedClock
from concourse.bass_utils import run_bass_kernel_spmd

B, C_IN, L = 32, 64, 8192
A, C_OUT, KTAPS = 64, 256, 9
NCORES = 8
BPC = B // NCORES          # batch rows per core
LPAD = L + 10              # 4 left pad + 5 right pad + 1 shift spare
NT = 512                   # L-tile columns (one PSUM bank)
NTILES = L // NT
NPAIRS = 4                 # taps 0-7 in pairs; tap 8 is folded into mm1
f32 = mybir.dt.float32
f32r = mybir.dt.float32r

USE_F32R = True


class _SplitDrainTileContext(tile.TileContext):
    """This walrus build rejects instructions carrying >1 sync wait. Split
    every multi-wait instruction's extra waits onto same-engine nops placed
    immediately before it (same-engine program order makes this equivalent),
    and do the same for the Tile epilogue drain."""

    def _split_multi_waits(self):
        nc = self.nc
        for fn in nc.m.functions:
            for bb in fn.blocks:
                insts = list(bb.instructions)
                out = []
                changed = False
                for inst in insts:
                    si = inst.sync_info
                    if si is not None and len(si.on_wait) > 1:
                        ow = list(si.on_wait)
                        for w in ow[:-1]:
                            nop = mybir.InstNoOp(
                                name=nc.get_next_instruction_name(),
                                engine=inst.engine,
                                sync_info=mybir.SyncInfo(
                                    on_wait=[w], on_update=[]
                                ),
                                bass_nofuse=True,
                            )
                            nc.register_instruction(nop)
                            out.append(nop)
                        si.on_wait = ow[-1:]
                        inst.sync_info = si
                        changed = True
                    out.append(inst)
                if changed:
                    bb.instructions = out

    def _drain_and_barrier(self, tick_clock, wait_clock):
        probe = self.nc.sync.nop(nofuse=True)
        wait_clock.add_sem_waits(
            probe.ins, ScopedClock({None: tick_clock.global_clock})
        )
        si = probe.ins.sync_info
        if si is not None:
            ow = list(si.on_wait)
            if len(ow) > 1:
                si.on_wait = ow[:1]
                probe.ins.sync_info = si
                for w in ow[1:]:
                    extra = self.nc.sync.nop(nofuse=True)
                    extra.ins.sync_info = mybir.SyncInfo(on_wait=[w], on_update=[])
        self.nc.sync.drain()
        self._split_multi_waits()

        self.nc.all_engine_barrier()
        assert self.sems is not None
        popped = self.nc._tile_sem_poison_stack.pop()
        assert popped is self._sem_poison
        self.nc.clear_and_free_semaphores(list(self.sems.allocated().values()))
        self.nc.all_engine_barrier()


def _mm(ap):
    return ap


def _build_nc(reps=1, mm_dt=f32r):
    nc = bass.Bass("TRN2", target_bir_lowering=False, debug=False,
                   num_devices=NCORES)
    xin = nc.dram_tensor("x", [BPC, C_IN, LPAD], mm_dt, kind="ExternalInput").ap()
    dl = nc.dram_tensor("dl", [128, NPAIRS * A], mm_dt, kind="ExternalInput").ap()
    w1t = nc.dram_tensor("w1t", [128, C_OUT], mm_dt, kind="ExternalInput").ap()
    w2t = nc.dram_tensor("w2t", [C_OUT, C_IN], mm_dt, kind="ExternalInput").ap()
    b1d = nc.dram_tensor("b1", [128, 2], f32, kind="ExternalInput").ap()
    b2d = nc.dram_tensor("b2", [C_IN, 1], f32, kind="ExternalInput").ap()
    y = nc.dram_tensor("y", [BPC, C_IN, L], f32, kind="ExternalOutput").ap()

    Relu = mybir.ActivationFunctionType.Relu
    Ident = mybir.ActivationFunctionType.Identity
    add = mybir.AluOpType.add
    amax = mybir.AluOpType.max

    with _SplitDrainTileContext(nc) as tc, ExitStack() as ctx:
        consts = ctx.enter_context(tc.tile_pool(name="consts", bufs=1))
        xrow_p = ctx.enter_context(tc.tile_pool(name="xrow", bufs=2))
        orow_p = ctx.enter_context(tc.tile_pool(name="orow", bufs=2))
        sp_p = ctx.enter_context(tc.tile_pool(name="sp", bufs=3))
        ft_p = ctx.enter_context(tc.tile_pool(name="ft", bufs=3))
        psA = ctx.enter_context(tc.tile_pool(name="psA", bufs=2, space="PSUM"))
        psB = ctx.enter_context(tc.tile_pool(name="psB", bufs=2, space="PSUM"))
        psC = ctx.enter_context(tc.tile_pool(name="psC", bufs=2, space="PSUM"))

        dl_sb = consts.tile([128, NPAIRS * A], mm_dt, tag="dl")
        nc.sync.dma_start(dl_sb[:], dl[:])
        w1t_sb = consts.tile([128, C_OUT], mm_dt, tag="w1t")
        w2t_sb = consts.tile([128, 2 * C_IN], mm_dt, tag="w2t")
        b1_sb = consts.tile([128, 2], f32, tag="b1")
        b2_sb = consts.tile([C_IN, 1], f32, tag="b2")

        def load_tail_consts():
            nc.sync.dma_start(w1t_sb[:], w1t[:])
            nc.sync.dma_start(w2t_sb[:, 0:C_IN], w2t[0:128, :])
            nc.sync.dma_start(w2t_sb[:, C_IN:2 * C_IN], w2t[128:256, :])
            nc.sync.dma_start(b1_sb[:], b1d[:])
            nc.sync.dma_start(b2_sb[:], b2d[:])

        # software pipeline with a 1-tile skew: stage1(i) = conv+evac,
        # stage2(i) = mm1+relu+mm2+evac, issued as s1(i+1); s2(i)
        work = [(b, t) for b in range(BPC) for t in range(NTILES)]
        xts = {}
        orows = {}
        stage1_out = {}

        def load_row_chunks(b, cuts):
            # partitions 0-63: x_pad; 64-127: x_pad shifted left by 1.
            # The DRAM-side AP reads the row twice, offset by one element,
            # so all 16 SBUF ports engage. Progressive chunks let tile-0
            # compute start before the whole row lands.
            xt = xts[b]
            for c0, c1 in zip(cuts[:-1], cuts[1:]):
                src = bass.AP(xin.tensor, b * C_IN * LPAD + c0,
                              [[1, 2], [LPAD, C_IN], [1, c1 - c0]])
                nc.sync.dma_start(xt[:, c0:c1], src)

        def init_row(b):
            xts[b] = xrow_p.tile([128, LPAD], mm_dt, tag="xt",
                                 name=f"xt{len(xts)}_{b}")
            orows[b] = orow_p.tile([C_IN, L], f32, tag="orow",
                                   name=f"orow{id(orows)}_{len(orows)}_{b}")

        def stage1(i):
            b, t = work[i]
            if t == 0 and b not in xts:
                init_row(b)
                load_row_chunks(b, [0, NT + 10, 4 * NT + 10, 8 * NT + 10, 12 * NT + 10, LPAD - 1])
            xt = xts[b]
            l0 = NT * t
            cps = psA.tile([A, NT], f32, tag="cps")
            for p in range(NPAIRS):
                nc.tensor.matmul(
                    cps[:],
                    _mm(dl_sb[:, A * p:A * (p + 1)]),
                    _mm(xt[:, l0 + 2 * p:l0 + 2 * p + NT]),
                    start=(p == 0),
                    stop=(p == NPAIRS - 1),
                )
            sp = sp_p.tile([128, NT], mm_dt, tag="sp")
            nc.vector.tensor_copy(sp[0:A, :], cps[:])
            # tap-8 operand for the fused mm1: xt upper half holds x_pad
            # shifted by 1, so col l0+7 aligns with x_pad[:, l0+j+8]
            nc.gpsimd.tensor_copy(sp[A:128, :], xt[64:128, l0 + 7:l0 + 7 + NT])
            stage1_out[i] = sp

        def stage2(i):
            b, t = work[i]
            sp = stage1_out.pop(i)
            l0 = NT * t
            f0 = psB.tile([128, NT], f32, tag="f0")
            nc.tensor.matmul(f0[:], _mm(w1t_sb[:, 0:128]), _mm(sp[:]),
                             start=True, stop=True)
            f1 = psB.tile([128, NT], f32, tag="f1")
            nc.tensor.matmul(f1[:], _mm(w1t_sb[:, 128:256]), _mm(sp[:]),
                             start=True, stop=True)
            ft0 = ft_p.tile([128, NT], mm_dt, tag="ft0")
            nc.scalar.activation(ft0[:], f0[:], Relu, bias=b1_sb[:, 0:1])
            ft1 = ft_p.tile([128, NT], mm_dt, tag="ft1")
            nc.vector.tensor_scalar(ft1[:], f1[:], b1_sb[:, 1:2], 0.0, add, amax)
            ops = psC.tile([C_IN, NT], f32, tag="ops")
            nc.tensor.matmul(ops[:], _mm(w2t_sb[:, 0:C_IN]), _mm(ft0[:]),
                             start=True, stop=False)
            nc.tensor.matmul(ops[:], _mm(w2t_sb[:, C_IN:2 * C_IN]), _mm(ft1[:]),
                             start=False, stop=True)
            nc.scalar.activation(orows[b][:, l0:l0 + NT], ops[:], Ident,
                                 bias=b2_sb[:, 0:1])
            if (t + 1) % (NTILES // 4) == 0:
                q = (t + 1) // (NTILES // 4) - 1
                c0, c1 = q * (L // 4), (q + 1) * (L // 4)
                nc.gpsimd.dma_start(y[b, :, c0:c1], orows[b][:, c0:c1])

        n = len(work)
        # prologue: dl is already queued; get row 0's first tile in flight,
        # then the remaining consts, then the bulk of row 0
        init_row(0)
        load_row_chunks(0, [0, NT + 10])
        load_tail_consts()
        load_row_chunks(0, [NT + 10, 2 * NT + 10, 4 * NT + 10, 8 * NT + 10,
                            12 * NT + 10, LPAD - 1])
        for r in range(reps):
            if r > 0:
                xts.clear()
                orows.clear()
            stage1(0)
            for i in range(n):
                if i + 1 < n:
                    stage1(i + 1)
                stage2(i)

    return nc


_NC = None


def _get_nc():
    global _NC
    if _NC is None:
        _NC = _build_nc()
    return _NC


def _prep_inputs(x, dictionary, w1, b1, w2, b2, mm_np=np.float32):
    x_pad = np.zeros((B, C_IN, LPAD), dtype=np.float32)
    x_pad[:, :, 4:4 + L] = x

    dl = np.zeros((128, NPAIRS, A), dtype=np.float32)
    for p in range(NPAIRS):
        dl[0:64, p, :] = dictionary[:, :, 2 * p].T
        dl[64:128, p, :] = dictionary[:, :, 2 * p + 1].T
    dl = np.ascontiguousarray(dl.reshape(128, NPAIRS * A))

    # mm1 lhsT rows 0-63: w1^T over atoms; rows 64-127: (w1 @ D8)^T so the
    # 9th conv tap rides the mm1 contraction instead of its own conv pass
    w8 = w1.astype(np.float64) @ dictionary[:, :, 8].astype(np.float64)
    w1t = np.ascontiguousarray(
        np.concatenate([w1.T, w8.T.astype(np.float32)], axis=0))  # [128, C_OUT]
    w2t = np.ascontiguousarray(w2.T)                      # [C_OUT, C_IN]
    b1m = np.ascontiguousarray(b1.reshape(2, 128).T)      # [128, 2]
    b2m = np.ascontiguousarray(b2.reshape(C_IN, 1))

    shared = {"dl": dl.astype(mm_np), "w1t": w1t.astype(mm_np),
              "w2t": w2t.astype(mm_np), "b1": b1m, "b2": b2m}
    in_maps = []
    for c in range(NCORES):
        m = dict(shared)
        m["x"] = np.ascontiguousarray(x_pad[c * BPC:(c + 1) * BPC]).astype(mm_np)
        in_maps.append(m)
    return in_maps


def run(inputs, **kwargs):
    """Run on hardware; returns (out [B, C_IN, L], BassKernelResults)."""
    arrs = {k: np.asarray(v, dtype=np.float32) for k, v in inputs.items()}
    in_maps = _prep_inputs(arrs["x"], arrs["dictionary"], arrs["w1"],
                           arrs["b1"], arrs["w2"], arrs["b2"])
    res = run_bass_kernel_spmd(_get_nc(), in_maps,
                               core_ids=list(range(NCORES)), **kwargs)
    out = np.concatenate([res.results[c]["y"] for c in range(NCORES)], axis=0)
    return out, res


def kernel(**inputs):
    out, _ = run(inputs)
    return out
